# revision 22
# baseline (speedup 1.0000x reference)
"""CrossNetMix (DCN-V2 MoE cross-network) Trainium2 Bass kernel.

Math (per layer i, reference):
    v    = tanh(x_l @ V[i])      per expert      (B, E, R)
    c    = tanh(v @ C[i].T)      per expert      (B, E, R)
    u    = c @ U[i].T            per expert      (B, E, D)
    gate = softmax(x_l @ G.T)                    (B, E)
    x_l  = sum_e gate_e * x0 * (u_e + bias[i]) + x_l

Since softmax gates sum to 1 the update is the direct recurrence
    x_{l+1} = x_l + x0 * (umix_l + bias[i]),  umix = U_arr^T (gate256 * c)

Device layout: features on partitions, tokens on the free dim.  The host
pre-transposes each core's x slice to (D, Bc) bf16 so every DMA is
contiguous and the whole matmul chain (V -> C -> U) stays feature-major
with zero on-device transposes.  Gate softmax over the E=4 partition dim
uses tiny matmuls (ones(4,4) broadcast-sum, selector broadcast 4->256).

All tensors are bf16 (psum accumulation stays fp32), which halves DMA
and SBUF traffic and enables the DVE 2x perf mode for all-SBUF bf16
element-wise ops.  Engine balance per layer:
  PE:   glog(8) V(16) sum(1) C(2) sel(2) U(16) matmuls
  ACT:  tanh v(2), tanh c(2), exp(1), gate-broadcast psum->bf16 copy(2)
  Pool: tmp_m = (ups_m + bias) * x0_m   (scalar_tensor_tensor, 8)
  DVE:  recip, gate mul, cg mul(2), X_m += tmp_m (bf16 2x mode, 8)
"""

import numpy as np
import ml_dtypes

import concourse.bacc as bacc
import concourse.bass as bass
import concourse.mybir as mybir
import concourse.tile as tile
from concourse.bass_utils import run_bass_kernel_spmd

# Problem constants (hardcoded per contract).
B, D, LAYERS, E, R = 16384, 1024, 3, 4, 64
ER = E * R                  # 256
NCORES = 8
BC = B // NCORES            # 2048 tokens per core
NB = 512                    # token block = PSUM bank width (fp32)
KC = D // 128               # 8 feature chunks
F32 = mybir.dt.float32
BF16 = mybir.dt.bfloat16

AF = mybir.ActivationFunctionType
OP = mybir.AluOpType


# Per-chunk recurrence strategy for layers >= 1 (8 entries):
#   'S': S-form: eye-matmul folds S into psum (PE), single stt makes x_new
#        (DVE), ACT copies S_new out of psum (except last layer).
#   'D': X-form: stt makes tmp (DVE), x_new = x_l + tmp on DVE (bf16 2x).
#   'P': X-form with the add on the Pool engine.
STRAT = ("P", "P", "P", "P", "D", "D", "D", "D")


def _emit(tc, outT, xT, w1, gt, cw, ua, biasP, sel, onesE, eye, n_blocks,
          strat=STRAT):
    nc = tc.nc
    from contextlib import ExitStack

    with ExitStack() as ctx:
        ctx.enter_context(nc.allow_low_precision(
            reason="bf16 pipeline validated numerically at 7e-3 max-rel"))
        consts = ctx.enter_context(tc.tile_pool(name="consts", bufs=1))
        xin = ctx.enter_context(tc.tile_pool(name="xin", bufs=2))
        work = ctx.enter_context(tc.tile_pool(name="work", bufs=2))
        pps = ctx.enter_context(tc.tile_pool(name="pps", bufs=2, space="PSUM"))
        ppv = ctx.enter_context(tc.tile_pool(name="ppv", bufs=2, space="PSUM"))
        ppm = ctx.enter_context(tc.tile_pool(name="ppm", bufs=2, space="PSUM"))
        ppu = ctx.enter_context(tc.tile_pool(name="ppu", bufs=2, space="PSUM"))

        # ---- resident weights (all bf16) ----
        # Layer-0-critical tensors are loaded first (each as ONE strided
        # DMA), then the x0 loads for the first block pair are emitted by
        # the caller, then the layer-1/2 weights (overlapped with layer-0
        # compute).
        w1_r = w1.rearrange("l (k p) er -> p l k er", p=128)
        ua_r = ua.rearrange("l (c p) d -> p l c d", p=128)
        gt_r = gt.rearrange("(k p) e -> p k e", p=128)

        gt_all = consts.tile([128, KC, E], BF16, name="gt_all")
        nc.sync.dma_start(out=gt_all, in_=gt_r)
        gt_sb = [gt_all[:, k, :] for k in range(KC)]

        w1_t = [consts.tile([128, KC, ER], BF16, name=f"w1_{i}")
                for i in range(LAYERS)]
        ua_t = [consts.tile([128, 2, D], BF16, name=f"ua_{i}")
                for i in range(LAYERS)]
        cw_t = [consts.tile([128, 2, 128], BF16, name=f"cw_{i}")
                for i in range(LAYERS)]
        bias_t = [consts.tile([128, KC], F32, name=f"bias_{i}")
                  for i in range(LAYERS)]
        w1_sb = [[w1_t[i][:, k, :] for k in range(KC)] for i in range(LAYERS)]
        ua_sb = [[ua_t[i][:, c, :] for c in range(2)] for i in range(LAYERS)]
        cw_sb = [[cw_t[i][:, j, :] for j in range(2)] for i in range(LAYERS)]
        bias_sb = bias_t

        def load_layer_weights(i):
            # Weights ride the Activation-engine DMA queue so they don't
            # contend with the x0 loads on the SP queue.  w1 is loaded
            # per-k-chunk so the V matmuls can start incrementally.
            for k in range(KC):
                nc.scalar.dma_start(out=w1_t[i][:, k, :],
                                    in_=w1_r[:, i, k, :])
            nc.scalar.dma_start(
                out=cw_t[i],
                in_=cw[i].rearrange("j p q -> p j q"))
            nc.scalar.dma_start(
                out=bias_t[i],
                in_=biasP[i].rearrange("(m p) -> p m", p=128))
            nc.scalar.dma_start(out=ua_t[i], in_=ua_r[:, i])

        sel_sb = consts.tile([E, ER], BF16, name="sel")
        nc.scalar.dma_start(out=sel_sb, in_=sel)
        onesE_sb = consts.tile([E, E], BF16, name="onesE")
        nc.scalar.dma_start(out=onesE_sb, in_=onesE)
        eye_sb = consts.tile([128, 128], BF16, name="eye")
        nc.scalar.dma_start(out=eye_sb, in_=eye)
        load_layer_weights(0)

        xT_r = xT.rearrange("(k p) t -> p k t", p=128)
        outT_r = outT.rearrange("(m p) t -> p m t", p=128)

        # Per-block state: running x_l tile, x0 tile, S tiles per S-chunk.
        xl_t = [None] * n_blocks
        x0_t = [None] * n_blocks
        S_t = [dict() for _ in range(n_blocks)]
        st = [dict() for _ in range(n_blocks)]   # per-block stage scratch

        def load_x0(b):
            x0 = xin.tile([128, KC, NB], BF16, tag="x0", bufs=4, name=f"x0_{b}")
            for k in range(KC):
                nc.sync.dma_start(out=x0[:, k, :],
                                  in_=xT_r[:, k, b * NB:(b + 1) * NB])
            x0_t[b] = x0
            xl_t[b] = x0

        def part1(b, l):
            """glog + V matmuls (per-k interleaved for just-in-time x_l
            consumption), exp, softmax-sum, tanh(v), recip, gate."""
            xl = xl_t[b]
            s = st[b]
            glog = pps.tile([E, NB], F32, tag="small", name=f"glog{b}_{l}")
            vps = [ppv.tile([128, NB], F32, tag="vps", name=f"vps{b}_{l}_{j}")
                   for j in range(2)]
            for k in range(KC):
                nc.tensor.matmul(glog, gt_sb[k], xl[:, k, :],
                                 start=(k == 0), stop=(k == KC - 1),
                                 skip_group_check=True)
                for j in range(2):
                    nc.tensor.matmul(
                        vps[j],
                        w1_sb[l][k][:, j * 128:(j + 1) * 128],
                        xl[:, k, :],
                        start=(k == 0), stop=(k == KC - 1),
                        skip_group_check=True)
            expg = work.tile([E, NB], BF16, tag="expg", name=f"expg{b}_{l}")
            nc.scalar.activation(expg, glog, AF.Exp)
            sumb = pps.tile([E, NB], F32, tag="small", name=f"sumb{b}_{l}")
            nc.tensor.matmul(sumb, onesE_sb, expg, start=True, stop=True)
            v_sb = [work.tile([128, NB], BF16, tag=f"vsb{j}",
                              name=f"vsb{b}_{l}_{j}") for j in range(2)]
            for j in range(2):
                nc.scalar.activation(v_sb[j], vps[j], AF.Tanh)
            recip = work.tile([E, NB], BF16, tag="recip", name=f"recip{b}_{l}")
            nc.vector.reciprocal(recip, sumb)
            gate = work.tile([E, NB], BF16, tag="gate", name=f"gate{b}_{l}")
            nc.vector.tensor_mul(gate, expg, recip)
            s["v_sb"], s["gate"] = v_sb, gate

        def part2(b, l):
            """C matmuls, gate broadcast, cg, U matmuls + recurrence."""
            s = st[b]
            v_sb, gate = s["v_sb"], s["gate"]
            xl, x0 = xl_t[b], x0_t[b]
            last = l == LAYERS - 1
            l0 = l == 0

            cps = [ppm.tile([128, NB], F32, tag="mid", name=f"cps{b}_{l}_{j}")
                   for j in range(2)]
            for j in range(2):
                nc.tensor.matmul(cps[j], cw_sb[l][j], v_sb[j],
                                 start=True, stop=True)
            c_sb = [work.tile([128, NB], BF16, tag=f"csb{j}",
                              name=f"csb{b}_{l}_{j}") for j in range(2)]
            for j in range(2):
                nc.scalar.activation(c_sb[j], cps[j], AF.Tanh)

            gps = [ppm.tile([128, NB], F32, tag="mid", name=f"gps{b}_{l}_{j}")
                   for j in range(2)]
            for j in range(2):
                nc.tensor.matmul(gps[j], sel_sb[:, j * 128:(j + 1) * 128],
                                 gate, start=True, stop=True)
            gb = [work.tile([128, NB], BF16, tag=f"gb{j}",
                            name=f"gb{b}_{l}_{j}") for j in range(2)]
            for j in range(2):
                nc.scalar.activation(gb[j], gps[j], AF.Identity)
            cg = [work.tile([128, NB], BF16, tag=f"cg{j}",
                            name=f"cg{b}_{l}_{j}") for j in range(2)]
            for j in range(2):
                nc.vector.tensor_mul(cg[j], c_sb[j], gb[j])

            # U matmuls + per-chunk recurrence.  At l0 every chunk is a
            # single stt: x_1 = x0*(1 + umix + bias0) (the +1 is folded
            # into biasP[0] on the host).  At l>=1 chunk strategy follows
            # `strat`.  All DVE stt's are emitted before any DVE adds so
            # the stt stream paces the next layer's matmul consumption.
            x_new = xin.tile([128, KC, NB], BF16, tag="X", bufs=4,
                             name=f"X{b}_{l}")
            dve_adds = []
            for m in range(KC):
                kind = "L0" if l0 else strat[m]
                ups = ppu.tile([128, NB], F32, tag="ups", name=f"ups{b}_{l}_{m}")
                for kc in range(2):
                    nc.tensor.matmul(
                        ups,
                        ua_sb[l][kc][:, m * 128:(m + 1) * 128],
                        cg[kc],
                        start=(kc == 0),
                        stop=(kc == 1 and kind != "S"))
                if kind == "S":
                    nc.tensor.matmul(ups, eye_sb, S_t[b][m],
                                     start=False, stop=True)
                bcol = bias_sb[l][:, m:m + 1]
                if kind in ("L0", "S"):
                    # x_new = (ups [+ S via eye] + bias) * x0, one stt
                    nc.vector.scalar_tensor_tensor(
                        out=x_new[:, m, :], in0=ups, scalar=bcol,
                        in1=x0[:, m, :], op0=OP.add, op1=OP.mult)
                    if (kind == "S" or strat[m] == "S") and not last:
                        S_new = work.tile([128, NB], BF16, tag=f"S{m}",
                                          name=f"S{b}_{l}_{m}")
                        nc.scalar.activation(S_new, ups, AF.Identity,
                                             bias=bcol)
                        S_t[b][m] = S_new
                else:
                    tmp = work.tile([128, NB], BF16, tag="tmp", bufs=4,
                                    name=f"tmp{b}_{l}_{m}")
                    nc.vector.scalar_tensor_tensor(
                        out=tmp, in0=ups, scalar=bcol, in1=x0[:, m, :],
                        op0=OP.add, op1=OP.mult)
                    if kind == "P":
                        nc.gpsimd.tensor_tensor(
                            out=x_new[:, m, :], in0=xl[:, m, :], in1=tmp,
                            op=OP.add)
                    else:
                        dve_adds.append((m, tmp))
                if last:
                    if kind in ("L0", "S") or kind == "P":
                        nc.scalar.dma_start(
                            out=outT_r[:, m, b * NB:(b + 1) * NB],
                            in_=x_new[:, m, :])
            for m, tmp in dve_adds:
                nc.vector.tensor_tensor(
                    out=x_new[:, m, :], in0=xl[:, m, :], in1=tmp, op=OP.add)
                if last:
                    nc.sync.dma_start(
                        out=outT_r[:, m, b * NB:(b + 1) * NB],
                        in_=x_new[:, m, :])
            xl_t[b] = x_new

        # ---- two-block interleaved pipeline ----
        # p1(b) and p2(b±1) overlap so each stage's gate-chain latency is
        # hidden by the sibling block's matmul phases.  Layer-1/2 weights
        # and the next pair's x0 are DMA'd during the first pair's compute.
        load_x0(0)
        load_x0(1)
        for pb in range(0, n_blocks, 2):
            b0, b1 = pb, pb + 1
            for l in range(LAYERS):
                part1(b0, l)
                if pb == 0 and l < LAYERS - 1:
                    load_layer_weights(l + 1)
                part1(b1, l)
                part2(b0, l)
                if l == 0:
                    for nb in (b0 + 2, b1 + 2):
                        if nb < n_blocks and x0_t[nb] is None:
                            load_x0(nb)
                part2(b1, l)


def build_bass(n_blocks=BC // NB, strat=STRAT):
    nc = bacc.Bacc(trn_type="TRN2", target_bir_lowering=False, debug=False)
    bc = n_blocks * NB

    def inp(name, shape, dt=BF16):
        return nc.dram_tensor(name, list(shape), dt, kind="ExternalInput").ap()

    xT = inp("xT", (D, bc))
    w1 = inp("w1", (LAYERS, D, ER))
    gt = inp("gt", (D, E))
    cw = inp("cw", (LAYERS, 2, 128, 128))
    ua = inp("ua", (LAYERS, ER, D))
    biasP = inp("biasP", (LAYERS, D), F32)
    sel = inp("sel", (E, ER))
    onesE = inp("onesE", (E, E))
    eye = inp("eye", (128, 128))
    outT = nc.dram_tensor("outT", [D, bc], BF16, kind="ExternalOutput").ap()

    with tile.TileContext(nc) as tc:
        _emit(tc, outT, xT, w1, gt, cw, ua, biasP, sel, onesE, eye, n_blocks,
              strat=strat)
    nc.compile()
    return nc


def prep_weights(U, V, C, bias, G):
    """Host-side weight rearrangement (replicated across cores)."""
    U = np.asarray(U, np.float32)
    V = np.asarray(V, np.float32)
    C = np.asarray(C, np.float32)
    bias = np.asarray(bias, np.float32)
    G = np.asarray(G, np.float32)
    bf = ml_dtypes.bfloat16

    # w1[i, d, e*R+r] = V[i, e, d, r]
    w1 = np.ascontiguousarray(
        V.transpose(0, 2, 1, 3).reshape(LAYERS, D, ER)).astype(bf)
    # ua[i, e*R+r, d] = U[i, e, d, r]
    ua = np.ascontiguousarray(
        U.transpose(0, 1, 3, 2).reshape(LAYERS, ER, D)).astype(bf)
    # block-diagonal C^T chunks: chunk j holds experts 2j, 2j+1
    cw = np.zeros((LAYERS, 2, 128, 128), np.float32)
    for i in range(LAYERS):
        for e in range(E):
            j, o = divmod(e, 2)
            cw[i, j, o * R:(o + 1) * R, o * R:(o + 1) * R] = C[i, e].T
    cw = cw.astype(bf)
    gt = np.ascontiguousarray(G.T).astype(bf)
    biasP = bias.copy()
    biasP[0] += 1.0  # x_1 = x0*(1 + umix + bias0); S_0 = 1 folded in
    sel = np.zeros((E, ER), np.float32)
    for e in range(E):
        sel[e, e * R:(e + 1) * R] = 1.0
    sel = sel.astype(bf)
    onesE = np.ones((E, E), bf)
    eye = np.eye(128, dtype=bf)
    return dict(w1=w1, gt=gt, cw=cw, ua=ua, biasP=biasP, sel=sel,
                onesE=onesE, eye=eye)


_NC_CACHE = {}


def _get_nc(n_blocks):
    if n_blocks not in _NC_CACHE:
        _NC_CACHE[n_blocks] = build_bass(n_blocks)
    return _NC_CACHE[n_blocks]


def run(inputs, trace=False, **spmd_kwargs):
    """Shard, run on 8 cores, gather.  Returns (output, BassKernelResults)."""
    x = np.asarray(inputs["x"], np.float32)
    weights = prep_weights(inputs["U"], inputs["V"], inputs["C"],
                           inputs["bias"], inputs["G"])
    nc = _get_nc(BC // NB)
    bf = ml_dtypes.bfloat16

    in_maps = []
    for c in range(NCORES):
        xc = np.ascontiguousarray(x[c * BC:(c + 1) * BC].T).astype(bf)
        in_maps.append(dict(xT=xc, **weights))

    res = run_bass_kernel_spmd(nc, in_maps, core_ids=list(range(NCORES)),
                               trace=trace, **spmd_kwargs)

    out = np.empty((B, D), np.float32)
    for c in range(NCORES):
        out[c * BC:(c + 1) * BC] = res.results[c]["outT"].astype(np.float32).T
    return out, res


def kernel(**inputs):
    out, _ = run(inputs)
    return out
